# revision 12
# baseline (speedup 1.0000x reference)
"""DeepInterestNet attention block on 8 Trainium2 NeuronCores.

The reference applies softmax over a size-1 axis, which yields all-ones
regardless of the attention-MLP scores, so the module's output reduces
exactly to an embedding-bag:

    out[b, :] = sum_l emb_table[seq_fea[b, l], :] * seq_fea_val[b, l]

Sharding: data-parallel over batch. Each of the 8 cores handles 128 batch
rows; the embedding table is replicated.

Gather strategy: the bulk-gather primitive (dma_gather) takes int16 row
indices, so the 100k-row table is processed as 4 range-shards of 25000
rows. On the host each batch row's 200 (idx, val) pairs are partitioned by
shard and padded to a fixed K=80 entries per shard (pad: local idx 0 with
val 0). Each shard is gathered with dma_gather (dest slot for stream
position i is partition i%128 = batch row, column i//128), multiplied by
val broadcast over E on the vector engine (written e-major), reduced over
the slot axis, and the 4+ partials are summed and stored.
"""
import sys

if '/opt/trn_rl_repo' not in sys.path:
    sys.path.insert(0, '/opt/trn_rl_repo')

import numpy as np

B, L, E, V = 1024, 200, 64, 100000
NCORES = 8
PB = B // NCORES          # 128 batch rows per core (partition dim)
NSHARD = 4
SHARD = V // NSHARD       # 25000 rows per shard (int16-addressable)
K = 80                    # padded entries per (row, shard); data max is 70
HALVES = 2                # split each shard gather for DMA/compute overlap
KH = K // HALVES          # 40 columns per gather call
NI = PB * KH              # num_idxs per dma_gather call (5120)

_cache = {}


def _build():
    import concourse.bass as bass
    import concourse.mybir as mybir
    from concourse import bacc
    from concourse.tile import TileContext

    nc = bacc.Bacc("TRN2", target_bir_lowering=False)
    # wrapped int16 index streams: one [PB, K*PB/16] tensor per shard
    idxs = [
        nc.dram_tensor(f"idxs{q}", [PB, K * PB // 16], mybir.dt.int16,
                       kind="ExternalInput")
        for q in range(NSHARD)
    ]
    val = nc.dram_tensor("val", [PB, NSHARD * K], mybir.dt.float32,
                         kind="ExternalInput")
    # one tensor per 25000-row table shard: the gather ucode requires a
    # zero AP offset on its source (nonzero bases read wrong rows)
    tables = [
        nc.dram_tensor(f"table{q}", [SHARD, E], mybir.dt.float32,
                       kind="ExternalInput")
        for q in range(NSHARD)
    ]
    out = nc.dram_tensor("out", [PB, E], mybir.dt.float32,
                         kind="ExternalOutput")

    with TileContext(nc) as tc:
        with tc.tile_pool(name="const", bufs=1) as cpool, \
             tc.tile_pool(name="gather", bufs=4) as gpool, \
             tc.tile_pool(name="wbuf", bufs=2) as wpool, \
             tc.tile_pool(name="acc", bufs=1) as apool:
            idx_ts = []
            for q in range(NSHARD):
                t = cpool.tile([PB, K * PB // 16], mybir.dt.int16,
                               tag=f"idx{q}")
                nc.sync.dma_start(out=t[:], in_=idxs[q][:, :])
                idx_ts.append(t)
            val_t = cpool.tile([PB, NSHARD * K], mybir.dt.float32)
            nc.sync.dma_start(out=val_t[:], in_=val[:, :])
            # DVE probe read of val_t: the vector engine observes the val
            # DMA semaphore here, so later multiplies carry only their
            # gather-semaphore wait.
            val_probe = cpool.tile([PB, 1], mybir.dt.float32)
            nc.vector.tensor_copy(out=val_probe[:], in_=val_t[:, :1])

            partials = []
            wcols_half = KH * PB // 16   # wrapped idx columns per half
            for q in range(NSHARD):
                for h in range(HALVES):
                    g = gpool.tile([PB, KH * E], mybir.dt.float32)
                    g3 = g[:].rearrange("p (s e) -> p s e", e=E)
                    nc.gpsimd.dma_gather(
                        g3,
                        tables[q][:, :],
                        idx_ts[q][:, h * wcols_half:(h + 1) * wcols_half],
                        num_idxs=NI,
                        num_idxs_reg=NI,
                        elem_size=E,
                        single_packet=False,
                    )
                    csl = slice((q * HALVES + h) * KH,
                                (q * HALVES + h + 1) * KH)
                    vb = val_t[:, csl].unsqueeze(2).broadcast_to([PB, KH, E])
                    w = wpool.tile([PB, KH * E], mybir.dt.float32)
                    # write e-major so the reduce below runs unit-stride
                    wt = w[:].rearrange("p (e s) -> p s e", s=KH)
                    nc.vector.tensor_tensor(out=wt, in0=g3, in1=vb,
                                            op=mybir.AluOpType.mult)
                    part = apool.tile([PB, E], mybir.dt.float32,
                                      tag=f"part{q}_{h}")
                    nc.vector.tensor_reduce(
                        out=part[:],
                        in_=w[:].rearrange("p (e s) -> p e s", s=KH),
                        axis=mybir.AxisListType.X,
                        op=mybir.AluOpType.add)
                    partials.append(part)

            while len(partials) > 1:
                nxt = []
                for k in range(0, len(partials) - 1, 2):
                    nc.vector.tensor_tensor(
                        out=partials[k][:], in0=partials[k][:],
                        in1=partials[k + 1][:], op=mybir.AluOpType.add)
                    nxt.append(partials[k])
                if len(partials) % 2:
                    nxt.append(partials[-1])
                partials = nxt
            nc.sync.dma_start(out=out[:, :], in_=partials[0][:])
    nc.compile()
    return nc


def _prep_host(seq_fea, seq_fea_val):
    """Sort each row's (idx, val) pairs into 4 shard segments padded to K.

    Returns (idx_wrapped [NSHARD, B, K*PB/16] int16, val_sorted [B, NSHARD*K]
    f32)."""
    idx = np.asarray(seq_fea).astype(np.int64, copy=False)
    val = np.asarray(seq_fea_val, dtype=np.float32)
    shard = idx // SHARD                               # [B, L]
    order = np.argsort(shard, axis=1, kind='stable')   # [B, L]
    rows = np.arange(B)[:, None]
    idx_s = idx[rows, order]                           # sorted by shard
    val_s = val[rows, order]
    shard_s = shard[rows, order]
    counts = np.zeros((B, NSHARD), np.int64)
    for q in range(NSHARD):
        counts[:, q] = (shard_s == q).sum(1)
    if counts.max() > K:
        raise ValueError(f"shard segment overflow: {counts.max()} > K={K}")
    starts = np.zeros((B, NSHARD), np.int64)
    starts[:, 1:] = np.cumsum(counts, axis=1)[:, :-1]
    # rank of each sorted entry within its shard segment
    rank = np.arange(L)[None, :] - starts[rows[:, :1] * 0 + rows, shard_s]
    col = shard_s * K + rank                           # [B, L] in [0, 4K)
    idx_pad = np.zeros((B, NSHARD * K), np.int16)
    val_pad = np.zeros((B, NSHARD * K), np.float32)
    flat = rows * (NSHARD * K) + col
    idx_pad.ravel()[flat.ravel()] = (idx_s - shard_s * SHARD).astype(np.int16).ravel()
    val_pad.ravel()[flat.ravel()] = val_s.ravel()

    # wrap each core-block/shard segment into dma_gather's int16 layout
    idx_wrapped = np.zeros((NSHARD, B, K * PB // 16), np.int16)
    for c in range(NCORES):
        blk = slice(c * PB, (c + 1) * PB)
        for q in range(NSHARD):
            seg = idx_pad[blk, q * K:(q + 1) * K]      # [PB, K]
            S = seg.T.ravel()                          # position j*PB+p
            w16 = S.reshape(-1, 16).T                  # [16, K*PB/16]
            idx_wrapped[q, blk] = np.tile(w16, (PB // 16, 1))
    return idx_wrapped, val_pad


def run_spmd(inputs, **spmd_kwargs):
    """Shard inputs, run the Bass kernel on 8 cores, gather the output."""
    from concourse.bass_utils import run_bass_kernel_spmd

    if 'nc' not in _cache:
        _cache['nc'] = _build()
    nc = _cache['nc']

    idx_wrapped, val_pad = _prep_host(inputs['seq_fea'],
                                      inputs['seq_fea_val'])
    table = np.asarray(inputs['emb_table'], dtype=np.float32)
    tshards = [np.ascontiguousarray(table[q * SHARD:(q + 1) * SHARD])
               for q in range(NSHARD)]

    in_maps = []
    for c in range(NCORES):
        blk = slice(c * PB, (c + 1) * PB)
        m = {'val': np.ascontiguousarray(val_pad[blk])}
        for q in range(NSHARD):
            m[f'table{q}'] = tshards[q]
            m[f'idxs{q}'] = np.ascontiguousarray(idx_wrapped[q, blk])
        in_maps.append(m)
    res = run_bass_kernel_spmd(nc, in_maps, core_ids=list(range(NCORES)),
                               **spmd_kwargs)
    full = np.concatenate(
        [np.asarray(res.results[c]['out']) for c in range(NCORES)], axis=0)
    return full, res


def kernel(**inputs):
    out, _ = run_spmd(inputs)
    return out
